# revision 71
# baseline (speedup 1.0000x reference)
"""Trainium2 Bass kernel for GQA causal attention block (B=2,T=2048,D=2048,H=16,G=4).

Sharding: 8 cores = batch(2) x kv-group(4). Core c handles batch b=c//4 and
kv-group g=c%4 (query heads 4g..4g+3, which share that kv group). Each core
computes a partial output y_g @ Wo[g-rows]; the host sums the 4 group
partials per batch in fp32 and applies the 1/D weight-scale folding.

v2: the four projections (Q/K/V/O) run as fp8-e4m3 DoubleRow matmuls with a
3-term hi/lo error split (hi@hi + lo@hi + hi@lo): 0.5 cycles/row per
256-deep contraction pair — 0.75x the bf16 cycle count with ~1.3e-3 output
error (validated on hw). The hi/lo splits of x.T and all weights are
precomputed on the host; weights are scaled by sqrt(D) so their entries sit
in e4m3's normal range (raw Wq sigma=D^-0.5 is subnormal in e4m3 and
quantizes catastrophically). The inverse scales fold into the exp scale
(1/D) and a final host-side 1/D multiply. ysb's hi/lo pair is produced
on-device (ACT copy + DVE subtract). Attention (QK, AV, den) stays fp16:
an un-split fp8 operand injects >=2.6e-2 relative error, over the 2e-2
gate. All former bf16 tensors are fp16 (same engine cost, 8x lower noise
floor).

Per-core dataflow (fp32 PSUM accumulation):
  xh/xl  = host-split x.T e4m3 hi/lo, plain linear DMA loads [d=128, o, t]
  QT_h = wq.T @ x.T  (PE DR split3, accum over d)  [dk=128, t] fp16
  KT   = wk.T @ x.T  (DR split3)                   [dk=128, t] fp16
  V    = x @ wv      (DR split3, natural)          [t=128-blk, dk] fp16
  RoPE: Q batched over the 4 heads; K per slice (fp16 DVE).
  per qslice j (512 queries), head h, key block tkb<=4j+3:
    ST  = KT_blk.T-contraction QK matmul -> PSUM [tk=128, tq=512] fp16 ops
    PT  = exp(scale*ST) on ACT -> SBUF fp16; diag blocks masked by 0/1 mult
          (alternating DVE/Pool)
    yt += V_blk.T @ PT   (PE accum)          [dk=128, tq=512]
  den = ones128.T @ tree_sum(PT blocks) (DVE tree, 1 PE matmul)
  ysb = yt * recip(den) (DVE) fp16; yhi = e4m3 (ACT copy);
  ylo = ysb - yhi (DVE) e4m3
  out[tq,:] += (partial) sum_h ysb_h.T @ wo_h  (PE DR split3 over head
  pairs, accum over terms; fp16 out, D scale folded out on host)

Schedule: token slice 0's projections run upfront with column-priority
DMA loads (ts0 x columns + wq stream in first, so the PE starts ~2us in).
Attention j's blocks are the emission spine; a pump drains filler units
between blocks — slice j+1's Q/rope (and K/V, deferred into early j+1
where legal since j+1 reads those K/V blocks only in its last pairs) and
wo_stage(j-1) groups — so the in-order PE stream always has independent
work while the exp chain trails, and ACT/exp work spreads over the whole
timeline. Per-head finalize (den matmul / recip / ysb / e4m3 hi-lo) is
deferred two blocks into the next head (and across the slice boundary
for the last head) so it never fills the 4-deep PE wait queue; the very
last finalize runs in column halves, with its den accumulated directly
over the ptq partial sums, so wo_final's first groups start sooner.

Error budget spend: the Q projection drops 3 of its 16 (term,
chunk-pair) corrections (lh pair 7, hl pairs 5-6) — each dropped pair
re-admits ~0.9e-2 of independent quantization noise (quadrature-
additive, model matches measurement within 5%) and saves 1.7us of PE.
Total rel err 1.58e-2 vs the 2e-2 gate (21% margin).

Engine balance: PE ~159us busy / 172.6us span, ACT ~132 (exp +
projection-PSUM copies + yhi), DVE ~126 (rope, tree-sums, mask-evens,
recip/ysb, wo copies), Pool ~88 (mask-odds, level-1 tree head + j2/j3
extras, ylo, j3 yhi, DMA queues). The ACT exp table is pre-warmed
during the projection phase.
"""

import sys
from contextlib import ExitStack

import numpy as np

sys.path.insert(0, "/opt/trn_rl_repo")

import ml_dtypes

import bass_rust
import concourse.bass as bass
import concourse.mybir as mybir
import concourse.tile as tile
from concourse.bass_utils import run_bass_kernel_spmd

B, T, D = 2, 2048, 2048
H, G, DK = 16, 4, 128
HPC = H // G          # 4 query heads per core
P = 128
NDC = D // P          # 16 contraction chunks
NPR = NDC // 2        # 8 DoubleRow chunk pairs
NTB = T // P          # 16 token blocks
QS = 512              # query slice (matmul moving dim)
NQS = T // QS         # 4
ND = D // QS          # 4 output column slices
THETA = 10000.0
SW = float(np.sqrt(D))                    # weight pre-scale (host)
SCALE = 1.0 / (float(np.sqrt(DK)) * D)    # exp scale: 1/sqrt(dk) / SW^2
OUTSCALE = 1.0 / D                        # host-side final unscale (SW^2)
F16 = mybir.dt.float16
E4 = mybir.dt.float8e4
F32 = mybir.dt.float32
DR = mybir.MatmulPerfMode.DoubleRow

_CACHE = {}
_NSPLIT = [0]


def split_multi_waits(nc):
    """Walrus codegen accepts at most one sem wait per instruction; Tile's
    sem assignment can emit several. Hoist extras onto single-wait NOPs
    inserted immediately before, on the same engine stream."""
    n = 0
    for f in nc.m.functions:
        for b in f.blocks:
            insts = b.instructions
            newl = []
            changed = False
            for ins in insts:
                si = getattr(ins, "sync_info", None)
                if si is not None and si.on_wait and len(si.on_wait) > 1:
                    waits = list(si.on_wait)
                    for w in waits[:-1]:
                        _NSPLIT[0] += 1
                        nop = bass_rust.InstNoOp(
                            name=f"I-wsplit{_NSPLIT[0]}",
                            engine=ins.engine,
                            ins=[], outs=[],
                            bass_nofuse=True,
                            sync_info=mybir.SyncInfo(on_wait=[w], on_update=[]),
                        )
                        newl.append(nop)
                        n += 1
                    ins.sync_info = mybir.SyncInfo(
                        on_wait=[waits[-1]], on_update=list(si.on_update or [])
                    )
                    changed = True
                newl.append(ins)
            if changed:
                insts.clear()
                insts.extend(newl)
    return n


def build_nc():
    nc = bass.Bass()
    xh = nc.declare_dram_parameter("xh", [D, T], E4, isOutput=False)
    xl = nc.declare_dram_parameter("xl", [D, T], E4, isOutput=False)
    wqh = nc.declare_dram_parameter("wqh", [D, HPC * DK], E4, isOutput=False)
    wql = nc.declare_dram_parameter("wql", [D, HPC * DK], E4, isOutput=False)
    wkh = nc.declare_dram_parameter("wkh", [D, DK], E4, isOutput=False)
    wkl = nc.declare_dram_parameter("wkl", [D, DK], E4, isOutput=False)
    wvh = nc.declare_dram_parameter("wvh", [D, DK], E4, isOutput=False)
    wvl = nc.declare_dram_parameter("wvl", [D, DK], E4, isOutput=False)
    woh = nc.declare_dram_parameter("woh", [HPC * DK, D], E4, isOutput=False)
    wol = nc.declare_dram_parameter("wol", [HPC * DK, D], E4, isOutput=False)
    cosf = nc.declare_dram_parameter("cosf", [P, T], F16, isOutput=False)
    sinf = nc.declare_dram_parameter("sinf", [P, T], F16, isOutput=False)
    dmask = nc.declare_dram_parameter("dmask", [HPC, P, QS], F16,
                                      isOutput=False)
    out = nc.declare_dram_parameter("out", [T, D], F16, isOutput=True)

    with ExitStack() as ctx:
        tc = ctx.enter_context(tile.TileContext(nc))
        const = ctx.enter_context(tc.tile_pool(name="const", bufs=1))
        work = ctx.enter_context(tc.tile_pool(name="work", bufs=3))
        ptp = ctx.enter_context(tc.tile_pool(name="ptp", bufs=8))
        pos_ = ctx.enter_context(tc.tile_pool(name="pos_", bufs=6))
        pst = ctx.enter_context(tc.tile_pool(name="pst", bufs=3, space="PSUM"))
        pyt = ctx.enter_context(tc.tile_pool(name="pyt", bufs=2, space="PSUM"))
        pden = ctx.enter_context(tc.tile_pool(name="pden", bufs=1, space="PSUM"))
        pmm = ctx.enter_context(tc.tile_pool(name="pmm", bufs=2, space="PSUM"))

        # ---- persistent SBUF loads ----
        # Queue spreading: SP carries xh (first matmul dep) then wo, ACT
        # carries xl, Pool carries the other weights+tables in need-order.
        xh_sb = const.tile([P, NDC, T], E4, tag="xh")
        xl_sb = const.tile([P, NDC, T], E4, tag="xl")
        wqh_sb = const.tile([P, NDC, HPC * DK], E4, tag="wqh")
        wql_sb = const.tile([P, NDC, HPC * DK], E4, tag="wql")
        wkh_sb = const.tile([P, NDC, DK], E4, tag="wkh")
        wkl_sb = const.tile([P, NDC, DK], E4, tag="wkl")
        wvh_sb = const.tile([P, NDC, DK], E4, tag="wvh")
        wvl_sb = const.tile([P, NDC, DK], E4, tag="wvl")
        xh_r = xh.rearrange("(o p) t -> p o t", p=P)
        xl_r = xl.rearrange("(o p) t -> p o t", p=P)
        # Column-priority loads: ts0's x columns (0:512, all 16 chunks) land
        # first in three transfers per tensor so the PE starts ~2us in; the
        # remaining columns stream behind. Queue split: SP=xh(+wo later),
        # ACT=xl, Pool=weights/tables in first-use order.
        wqh_r = wqh.rearrange("(o p) m -> p o m", p=P)
        wql_r = wql.rearrange("(o p) m -> p o m", p=P)
        # Column-priority loads: ts0's x columns (0:512, all 16 chunks)
        # land early so the PE starts ~2.5us in; wq (first consumer) heads
        # the Pool queue in hi/lo chunk-priority order.
        nc.gpsimd.dma_start(wqh_sb[:, :4, :], wqh_r[:, :4, :])
        nc.gpsimd.dma_start(wqh_sb[:, 4:, :], wqh_r[:, 4:, :])
        nc.sync.dma_start(xh_sb[:, :4, :QS], xh_r[:, :4, :QS])
        nc.scalar.dma_start(xl_sb[:, :4, :QS], xl_r[:, :4, :QS])
        nc.sync.dma_start(xh_sb[:, 4:, :QS], xh_r[:, 4:, :QS])
        nc.scalar.dma_start(wql_sb[:, :4, :], wql_r[:, :4, :])
        nc.scalar.dma_start(xl_sb[:, 4:, :QS], xl_r[:, 4:, :QS])
        nc.sync.dma_start(wql_sb[:, 4:, :], wql_r[:, 4:, :])
        nc.sync.dma_start(xh_sb[:, :, QS:], xh_r[:, :, QS:])
        nc.scalar.dma_start(xl_sb[:, :, QS:], xl_r[:, :, QS:])
        nc.gpsimd.dma_start(wkh_sb[:], wkh.rearrange("(o p) m -> p o m", p=P))
        nc.gpsimd.dma_start(wkl_sb[:], wkl.rearrange("(o p) m -> p o m", p=P))
        nc.gpsimd.dma_start(wvh_sb[:], wvh.rearrange("(o p) m -> p o m", p=P))
        nc.gpsimd.dma_start(wvl_sb[:], wvl.rearrange("(o p) m -> p o m", p=P))
        # cos/sin feed the ts0 rope; mask feeds attention j0
        cos_sb = const.tile([P, T], F16, tag="cos")
        nc.gpsimd.dma_start(cos_sb[:], cosf[:])
        sin_sb = const.tile([P, T], F16, tag="sin")
        nc.gpsimd.dma_start(sin_sb[:], sinf[:])
        mask_sb = const.tile([P, HPC, QS], F16, tag="mask")
        nc.gpsimd.dma_start(mask_sb[:], dmask.rearrange("d p q -> p d q"))
        # wo isn't read until the first wo_stage (~60% in): ride SP after xh
        woh_sb = const.tile([P, HPC, D], E4, tag="woh")
        nc.sync.dma_start(woh_sb[:], woh.rearrange("(h p) n -> p h n", p=P))
        wol_sb = const.tile([P, HPC, D], E4, tag="wol")
        nc.sync.dma_start(wol_sb[:], wol.rearrange("(h p) n -> p h n", p=P))
        ones_sb = const.tile([P, P], F16, tag="ones")
        nc.vector.memset(ones_sb[:], 1.0)
        # zero-init the pt pool slots: diagonal blocks only exp the unmasked
        # columns, and mask*stale-NaN would poison the sums otherwise
        for i in range(12):
            ptz = ptp.tile([P, QS], F16, tag="pt", name=f"ptz{i}", bufs=12)
            nc.vector.memset(ptz[:], 0.0)
        # warm the ACT exp table during the (ACT-idle) projection phase
        warm = work.tile([P, 1], F32, tag="warm", name="warm")
        nc.vector.memset(warm[:], 0.0)
        nc.scalar.activation(warm[:], warm[:],
                             mybir.ActivationFunctionType.Exp)

        # ---- projections (sliced, interleaved with attention) ----
        _pp = [(pmm, "mm"), (pst, "st"), (pyt, "yt"), (pden, "den")]
        _pg = [0]

        def proj_psum(cyc):
            if cyc:
                pool, tg = _pp[_pg[0] % 4]
                _pg[0] += 1
            else:
                pool, tg = pmm, "mm"
            return pool.tile([P, QS], F32, tag=tg, name=f"pj{_pg[0]}_{cyc}")

        def dr_term(ps, csl, w_ap, x_ap, start, stop):
            """One hi/lo term: 8 DoubleRow chunk-pair matmuls into ps[:, csl].
            w_ap: [P, NDC, M<=128] stationary; x_ap: [P, NDC, N<=256]
            moving (column slice pre-applied)."""
            for pr in range(NPR):
                nc.tensor.matmul(
                    ps[:, csl],
                    w_ap[:, 2 * pr:2 * pr + 2, :],
                    x_ap[:, 2 * pr:2 * pr + 2, :],
                    start=(start and pr == 0),
                    stop=(stop and pr == NPR - 1),
                    perf_mode=DR,
                )

        def dr3(ps, csl, wh_ap, wl_ap, xh_ap, xl_ap, skip_lh=(),
                skip_hl=()):
            """3-term hi/lo DoubleRow projection into ps[:, csl].
            skip_lh/skip_hl drop the correction terms for those chunk
            pairs: each dropped pair adds ~1e-2*sqrt(1/8) relative noise
            to this projection's output — spent deliberately from the
            error budget (gate 2e-2, floor 2.5e-3) to save PE cycles."""
            plan = [(wh_ap, xh_ap, pr) for pr in range(NPR)]
            plan += [(wl_ap, xh_ap, pr) for pr in range(NPR)
                     if pr not in skip_lh]
            plan += [(wh_ap, xl_ap, pr) for pr in range(NPR)
                     if pr not in skip_hl]
            for i, (w_ap, x_ap, pr) in enumerate(plan):
                nc.tensor.matmul(
                    ps[:, csl],
                    w_ap[:, 2 * pr:2 * pr + 2, :],
                    x_ap[:, 2 * pr:2 * pr + 2, :],
                    start=(i == 0), stop=(i == len(plan) - 1),
                    perf_mode=DR,
                )

        def rope_slice(dst, ts, nm):  # dst: [128, 512] f16 AP, token slice ts
            sl = slice(ts * QS, (ts + 1) * QS)
            sw = work.tile([P, QS], F16, tag="swp", name=f"sw{nm}")
            nc.gpsimd.dma_start(sw[0:64, :], dst[64:128, :])
            nc.gpsimd.dma_start(sw[64:128, :], dst[0:64, :])
            nc.vector.tensor_mul(sw[:], sw[:], sin_sb[:, sl])
            nc.vector.tensor_mul(dst, dst, cos_sb[:, sl])
            nc.vector.tensor_add(dst, dst, sw[:])

        QT = const.tile([P, HPC, T], F16, tag="QT")
        KT = const.tile([P, T], F16, tag="KT")
        Vn = const.tile([P, NTB, DK], F16, tag="Vn")

        def proj_q_units(h, ts, cyc=False):
            # two ~1.3us PE units sharing one PSUM tile (kept adjacent in
            # the filler list so the pmm ring never clobbers an open group)
            hs = slice(h * DK, (h + 1) * DK)
            t0 = ts * QS
            box = [None]

            def unit_a():
                box[0] = proj_psum(cyc)
                dr3(box[0], slice(0, 256),
                    wqh_sb[:, :, hs], wql_sb[:, :, hs],
                    xh_sb[:, :, t0:t0 + 256], xl_sb[:, :, t0:t0 + 256],
                    skip_lh=(7,), skip_hl=(5, 6))

            def unit_b():
                dr3(box[0], slice(256, 512),
                    wqh_sb[:, :, hs], wql_sb[:, :, hs],
                    xh_sb[:, :, t0 + 256:t0 + QS],
                    xl_sb[:, :, t0 + 256:t0 + QS],
                    skip_lh=(7,), skip_hl=(5, 6))
                nc.scalar.copy(QT[:, h, t0:t0 + QS], box[0][:])

            return [unit_a, unit_b]

        def rope_q4(ts):
            # batched rope over all 4 heads of a query slice
            sl = slice(ts * QS, (ts + 1) * QS)
            qs = QT[:, :, sl]
            sw = work.tile([P, HPC, QS], F16, tag="sw4", name=f"sw4_{ts}", bufs=2)
            nc.gpsimd.dma_start(sw[0:64, :, :], QT[64:128, :, sl])
            nc.gpsimd.dma_start(sw[64:128, :, :], QT[0:64, :, sl])
            sinb = sin_sb[:, sl].rearrange(
                "p (o c) -> p o c", o=1).broadcast_to((P, HPC, QS))
            cosb = cos_sb[:, sl].rearrange(
                "p (o c) -> p o c", o=1).broadcast_to((P, HPC, QS))
            nc.vector.tensor_mul(sw[:], sw[:], sinb)
            nc.vector.tensor_mul(qs, qs, cosb)
            nc.vector.tensor_add(qs, qs, sw[:])

        def proj_k_units(ts, cyc=False):
            t0 = ts * QS
            box = [None]

            def unit_a():
                box[0] = proj_psum(cyc)
                dr3(box[0], slice(0, 256), wkh_sb, wkl_sb,
                    xh_sb[:, :, t0:t0 + 256], xl_sb[:, :, t0:t0 + 256])

            def unit_b():
                dr3(box[0], slice(256, 512), wkh_sb, wkl_sb,
                    xh_sb[:, :, t0 + 256:t0 + QS],
                    xl_sb[:, :, t0 + 256:t0 + QS])
                nc.scalar.copy(KT[:, t0:t0 + QS], box[0][:])
                rope_slice(KT[:, t0:t0 + QS], ts, f"k{ts}")

            return [unit_a, unit_b]

        def proj_v_unit(tb, cyc=False):
            # natural-layout V: lhsT = x chunk pairs (stationary),
            # rhs = wv chunk pairs; out [t-block 128, dk 128]
            def unit():
                ps = proj_psum(cyc)
                tsl = slice(tb * P, (tb + 1) * P)
                first = True
                for (xt_, wt) in ((xh_sb, wvh_sb), (xl_sb, wvh_sb),
                                  (xh_sb, wvl_sb)):
                    for pr in range(NPR):
                        nc.tensor.matmul(
                            ps[:, :DK],
                            xt_[:, 2 * pr:2 * pr + 2, tsl],
                            wt[:, 2 * pr:2 * pr + 2, :],
                            start=first,
                            stop=(xt_ is xh_sb and wt is wvl_sb
                                  and pr == NPR - 1),
                            perf_mode=DR,
                        )
                        first = False
                nc.scalar.copy(Vn[:, tb, :], ps[:, :DK])

            return unit

        def proj_slice_units(ts, cyc=False):
            """All projection work for token slice ts, as filler units.
"""
            units = []
            for h in range(HPC):
                units += proj_q_units(h, ts, cyc=cyc)
            units.append(lambda: rope_q4(ts))
            units += proj_k_units(ts, cyc=cyc)
            for tb in range(4 * ts, 4 * ts + 4):
                units.append(proj_v_unit(tb, cyc=cyc))
            return units

        # token slice 0 runs upfront (attention j0 needs it); slices j+1
        # are spread through attention j as PE filler. ts0's Q projection
        # is emitted TERM-major (all hh chains, then lh, then hl) across
        # the four cycled PSUM pools, so wqh/wql/xl are each needed as
        # late as possible while their DMAs stream in.
        for u in proj_slice_units(0, cyc=True):
            u()

        _oq = [0]

        def wo_group(j, yhi, ylo, tqb, ds, pool, tg, pw, off, cpe, eng,
                     osb_sh=None):
            # one output-projection group: DR split3 over head pairs
            # (contraction = 4 heads x 128 = 2 DoubleRow pairs x 3 terms,
            # per 256-col chunk), PSUM->SBUF copy, out-DMA
            r0 = j * QS + tqb * P
            tq = slice(tqb * P, (tqb + 1) * P)
            po = pool.tile([P, QS], F32, tag=tg,
                           name=f"po{j}_{tqb}_{ds}_{off}")
            for cc in range(pw // 256):
                pc0 = cc * 256
                wsl = slice(ds * QS + off + pc0, ds * QS + off + pc0 + 256)
                first = True
                for (yt_, wt) in ((yhi, woh_sb), (ylo, woh_sb),
                                  (yhi, wol_sb)):
                    for hp in range(HPC // 2):
                        nc.tensor.matmul(
                            po[:, pc0:pc0 + 256],
                            yt_[:, 2 * hp:2 * hp + 2, tq],
                            wt[:, 2 * hp:2 * hp + 2, wsl],
                            start=first,
                            stop=(yt_ is yhi and wt is wol_sb
                                  and hp == HPC // 2 - 1),
                            perf_mode=DR,
                        )
                        first = False
            if osb_sh is not None:
                osb = osb_sh[:, off:off + pw]
            else:
                osb = pos_.tile([P, QS], F16, tag="osb",
                                name=f"osb{j}_{tqb}_{ds}_{off}")
                osb = osb[:, :pw]
            if cpe is nc.scalar:
                nc.scalar.copy(osb[:], po[:, :pw])
            else:
                cpe.tensor_copy(osb[:], po[:, :pw])
            eng.dma_start(
                out[r0:r0 + P, ds * QS + off:ds * QS + off + pw], osb[:])

        def wo_units(j, yhi, ylo):
            units = []
            for tqb in range(QS // P):
                for ds in range(ND):
                    def unit(tqb=tqb, ds=ds):
                        cpe = nc.vector
                        eng = nc.sync if _oq[0] % 2 == 0 else nc.gpsimd
                        _oq[0] += 1
                        wo_group(j, yhi, ylo, tqb, ds, pmm, "mm",
                                 QS, 0, cpe, eng)
                    unit.is_wo = True
                    units.append(unit)
            return units

        def wo_final(j, yhi, ylo):
            # final stage: attention PSUM pools are free — cycle po across
            # all four; split the very last group into two half-width
            # pieces on different banks so the copy+DMA drain pipelines
            gi = 0
            for tqb in range(QS // P):
                for ds in range(ND):
                    last = tqb == QS // P - 1 and ds >= ND - 2
                    if not last:
                        pool, tg = _pp[gi % 4]
                        gi += 1
                        cpe = nc.scalar if gi % 2 == 0 else nc.vector
                        eng = nc.sync if _oq[0] % 2 == 0 else nc.gpsimd
                        _oq[0] += 1
                        wo_group(j, yhi, ylo, tqb, ds, pool, tg,
                                 QS, 0, cpe, eng)
                    else:
                        osb_sh = pos_.tile([P, QS], F16, tag="osb",
                                           name=f"osbshf{j}_{ds}")
                        for pc in range(2):
                            pool, tg = _pp[gi % 4]
                            gi += 1
                            wo_group(j, yhi, ylo, tqb, ds, pool, tg,
                                     256, pc * 256,
                                     [nc.scalar, nc.vector][pc],
                                     [nc.scalar, nc.sync][pc], osb_sh=osb_sh)

        def proj_kv_units(ts, cyc=False):
            units = proj_k_units(ts, cyc=cyc)
            for tb in range(4 * ts, 4 * ts + 4):
                units.append(proj_v_unit(tb, cyc=cyc))
            return units

        def proj_qr_units(ts, cyc=False):
            units = []
            for h in range(HPC):
                units += proj_q_units(h, ts, cyc=cyc)
            units.append(lambda: rope_q4(ts))
            return units

        ysbs = {}
        fin = [None]   # deferred per-head finalize (crosses slice bounds)
        # ---- attention + output projection, per query slice ----
        # Emission interleave: attention j's blocks are the spine; between
        # blocks a pump drains filler units (projection slice j+1 and
        # wo_stage(j-1) groups) so the in-order PE stream always has
        # independent work while the exp chain trails, and ACT/exp work is
        # spread over the whole timeline instead of piling up after the
        # projection phase.
        for j in range(NQS):
            prio = []   # must complete early: drained one per block
            if fin[0] is not None:
                prio.append(fin[0])
                fin[0] = None
            fillers = []
            if j == NQS - 1:
                prio += proj_kv_units(j)
            if j + 1 < NQS:
                fillers += proj_qr_units(j + 1)
                if j + 1 < NQS - 1:
                    fillers += proj_kv_units(j + 1)
            if j >= 1:
                fillers += wo_units(j - 1, *ysbs[j - 1])
            nunits = len(fillers)
            nblocks = HPC * (4 * j + 4)
            emitted = [0]

            def pump(bdone):
                if prio:
                    prio.pop(0)()
                want = nunits * bdone // nblocks
                while emitted[0] < want and fillers:
                    fillers.pop(0)()
                    emitted[0] += 1

            ysb = work.tile([P, HPC, QS], F16, tag="ysb", bufs=2)
            yhi = work.tile([P, HPC, QS], E4, tag="yhi", bufs=2)
            ylo = work.tile([P, HPC, QS], E4, tag="ylo", bufs=2)
            nkb = 4 * j + 4  # causal: key blocks 0..4j+3
            for h in range(HPC):
                yt = pyt.tile([P, QS], F32, tag="yt")
                prev_pt = None
                ptot = None
                ptqs = []
                for tkb in range(nkb):
                    pump(h * nkb + tkb)
                    if tkb == 2 and fin[0] is not None:
                        # emit the previous head's finalize a couple of
                        # blocks in, so its den-matmul + the new head's
                        # first AVs don't fill the 4-deep PE wait queue at
                        # the head boundary
                        fin[0]()
                        fin[0] = None
                    d = tkb - 4 * j
                    # columns left of 128*d are fully masked for diagonal
                    # blocks: skip them in QK/exp/AV; the mask-mult zeroes
                    # the stale region of pt so den/AV sums stay exact.
                    c0 = max(d, 0) * P
                    st = pst.tile([P, QS], F32, tag="st")
                    nc.tensor.matmul(
                        st[:, c0:],
                        KT[:, tkb * P:(tkb + 1) * P],
                        QT[:, h, j * QS + c0:(j + 1) * QS],
                        start=True, stop=True,
                    )
                    pt = ptp.tile([P, QS], F16, tag="pt", bufs=12)
                    nc.scalar.activation(
                        pt[:, c0:], st[:, c0:],
                        mybir.ActivationFunctionType.Exp, scale=SCALE,
                    )
                    if d >= 0:
                        # full-width 0/1 mask also zeroes the stale c0
                        # region; alternate DVE/Pool to balance load
                        if d % 2 == 0:
                            nc.vector.tensor_mul(pt[:], pt[:],
                                                 mask_sb[:, d, :])
                        else:
                            nc.gpsimd.tensor_mul(pt[:], pt[:],
                                                 mask_sb[:, d, :])
                    nc.tensor.matmul(
                        yt[:, c0:], Vn[:, tkb, :], pt[:, c0:],
                        start=(tkb == 0), stop=(tkb == nkb - 1),
                    )
                    # denominator: tree-sum all PT blocks on DVE, then one
                    # ones-matmul per (h, j) for the partition reduction
                    if tkb % 2 == 0:
                        prev_pt = pt
                    else:
                        pts = ptp.tile([P, QS], F16, tag="pts",
                                       name=f"pts{j}_{h}_{tkb}", bufs=4)
                        # first tree level alternates Pool/DVE: off-DVE adds
                        # keep pt-slot recycling from serializing behind the
                        # per-head finalize, while capping Pool load
                        if (tkb <= 3 or (j == NQS - 1 and tkb <= 7)
                                or (j == NQS - 2 and tkb <= 5)):
                            nc.gpsimd.tensor_add(pts[:], prev_pt[:], pt[:])
                        else:
                            nc.vector.tensor_add(pts[:], prev_pt[:], pt[:])
                        if tkb % 4 == 1:
                            prev_pts = pts
                        else:
                            ptq = ptp.tile([P, QS], F16, tag="ptq", bufs=5,
                                           name=f"ptq{j}_{h}_{tkb}")
                            nc.vector.tensor_add(ptq[:], prev_pts[:], pts[:])
                            if j == NQS - 1 and h == HPC - 1:
                                ptqs.append(ptq)
                            elif ptot is None:
                                ptot = ptq
                            else:
                                nxt = ptp.tile([P, QS], F16, tag="ptt",
                                               bufs=5,
                                               name=f"ptt{j}_{h}_{tkb}")
                                nc.vector.tensor_add(nxt[:], ptot[:], ptq[:])
                                ptot = nxt
                def finalize(h=h, yt=yt, ptot=ptot, ptqs=ptqs, j=j,
                             ysb=ysb, yhi=yhi, ylo=ylo):
                    den = pden.tile([P, QS], F32, tag="den",
                                    name=f"den{j}_{h}")
                    recipb = work.tile([P, QS], F32, tag="recipb",
                                       name=f"rb{j}_{h}", bufs=2)
                    lastf = j == NQS - 1 and h == HPC - 1
                    # the very last finalize feeds wo_final directly: run
                    # it in column halves so the first wo groups start
                    # sooner
                    halves = ((slice(0, 256), slice(256, QS)) if lastf
                              else (slice(0, QS),))
                    for cs in halves:
                        if ptqs:
                            for qi, pq in enumerate(ptqs):
                                nc.tensor.matmul(
                                    den[:, cs], ones_sb[:], pq[:, cs],
                                    start=(qi == 0),
                                    stop=(qi == len(ptqs) - 1))
                        else:
                            nc.tensor.matmul(den[:, cs], ones_sb[:],
                                             ptot[:, cs],
                                             start=True, stop=True)
                        nc.vector.reciprocal(recipb[:, cs], den[:, cs])
                        nc.vector.tensor_mul(ysb[:, h, cs], yt[:, cs],
                                             recipb[:, cs])
                        if lastf:
                            # short critical path into wo_final: keep DVE
                            nc.vector.tensor_copy(yhi[:, h, cs],
                                                  ysb[:, h, cs])
                        elif j == NQS - 1:
                            # j3: ACT is exp-bound and DVE is saturated
                            nc.gpsimd.tensor_copy(yhi[:, h, cs],
                                                  ysb[:, h, cs])
                        else:
                            nc.scalar.copy(yhi[:, h, cs], ysb[:, h, cs])
                        if lastf:
                            nc.vector.tensor_sub(ylo[:, h, cs],
                                                 ysb[:, h, cs],
                                                 yhi[:, h, cs])
                        else:
                            nc.gpsimd.tensor_sub(ylo[:, h, cs],
                                                 ysb[:, h, cs],
                                                 yhi[:, h, cs])

                fin[0] = finalize

            if j == NQS - 1 and fin[0] is not None:
                # last slice: wo_final needs the final head's outputs now
                fin[0]()
                fin[0] = None
            while fillers:   # drain leftovers before the next slice
                fillers.pop(0)()
            ysbs[j] = (yhi, ylo)
        wo_final(NQS - 1, *ysbs[NQS - 1])
    split_multi_waits(nc)
    return nc


def _rope_tables(pos):
    inv_freq = 1.0 / (THETA ** (np.arange(0, DK // 2, dtype=np.float64) * 2.0 / DK))
    ang = pos.astype(np.float64)[:, None] * inv_freq[None, :]   # (T, 64)
    cos = np.cos(ang).T.astype(np.float32)                      # (64, T)
    sin = np.sin(ang).T.astype(np.float32)
    cosf = np.concatenate([cos, cos], axis=0)                   # (128, T)
    sinf = np.concatenate([-sin, sin], axis=0)
    return cosf, sinf


def _e4_split(a):
    """Split fp32 array into e4m3 hi + lo (residual)."""
    e4 = ml_dtypes.float8_e4m3
    hi = a.astype(e4)
    lo = (a - hi.astype(np.float32)).astype(e4)
    return hi, lo


def _make_in_maps(inputs):
    x, Wq, Wk, Wv, Wo = (np.asarray(inputs[k], dtype=np.float32) for k in
                         ("x", "Wq", "Wk", "Wv", "Wo"))
    f16 = np.float16
    cosf, sinf = _rope_tables(np.asarray(inputs["pos"]))
    cosf = cosf.astype(f16)
    sinf = sinf.astype(f16)
    # diagonal-region 0/1 masks: dmask[d][tk, tq] = mask[tq, d*128 + tk]
    m = np.asarray(inputs["mask"])
    dmask = np.stack(
        [m[0:QS, d * P:(d + 1) * P].T for d in range(HPC)], axis=0
    ).astype(f16)

    in_maps = []
    for c in range(8):
        b, g = c // 4, c % 4
        xt = np.ascontiguousarray(x[b].T)
        xh_, xl_ = _e4_split(xt)
        wq_h, wq_l = _e4_split(Wq[:, g * HPC * DK:(g + 1) * HPC * DK] * SW)
        wk_h, wk_l = _e4_split(Wk[:, g * DK:(g + 1) * DK] * SW)
        wv_h, wv_l = _e4_split(Wv[:, g * DK:(g + 1) * DK] * SW)
        wo_h, wo_l = _e4_split(Wo[g * HPC * DK:(g + 1) * HPC * DK, :] * SW)
        in_maps.append({
            "xh": xh_, "xl": xl_,
            "wqh": wq_h, "wql": wq_l,
            "wkh": wk_h, "wkl": wk_l,
            "wvh": wv_h, "wvl": wv_l,
            "woh": wo_h, "wol": wo_l,
            "cosf": cosf, "sinf": sinf, "dmask": dmask,
        })
    return in_maps


def kernel(x, Wq, Wk, Wv, Wo, mask, pos):
    in_maps = _make_in_maps(dict(x=x, Wq=Wq, Wk=Wk, Wv=Wv, Wo=Wo,
                                 mask=mask, pos=pos))
    if "nc" not in _CACHE:
        _CACHE["nc"] = build_nc()
    nc = _CACHE["nc"]

    res = run_bass_kernel_spmd(nc, in_maps, core_ids=list(range(8)))
    outs = [np.asarray(r["out"], dtype=np.float32) for r in res.results]
    full = np.stack([
        outs[0] + outs[1] + outs[2] + outs[3],
        outs[4] + outs[5] + outs[6] + outs[7],
    ]).astype(np.float32) * np.float32(OUTSCALE)
    return full


# revision 72
# speedup vs baseline: 1.0002x; 1.0002x over previous
"""Trainium2 Bass kernel for GQA causal attention block (B=2,T=2048,D=2048,H=16,G=4).

Sharding: 8 cores = batch(2) x kv-group(4). Core c handles batch b=c//4 and
kv-group g=c%4 (query heads 4g..4g+3, which share that kv group). Each core
computes a partial output y_g @ Wo[g-rows]; the host sums the 4 group
partials per batch in fp32 and applies the 1/D weight-scale folding.

v2: the four projections (Q/K/V/O) run as fp8-e4m3 DoubleRow matmuls with a
3-term hi/lo error split (hi@hi + lo@hi + hi@lo): 0.5 cycles/row per
256-deep contraction pair — 0.75x the bf16 cycle count with ~1.3e-3 output
error (validated on hw). The hi/lo splits of x.T and all weights are
precomputed on the host; weights are scaled by sqrt(D) so their entries sit
in e4m3's normal range (raw Wq sigma=D^-0.5 is subnormal in e4m3 and
quantizes catastrophically). The inverse scales fold into the exp scale
(1/D) and a final host-side 1/D multiply. ysb's hi/lo pair is produced
on-device (ACT copy + DVE subtract). Attention (QK, AV, den) stays fp16:
an un-split fp8 operand injects >=2.6e-2 relative error, over the 2e-2
gate. All former bf16 tensors are fp16 (same engine cost, 8x lower noise
floor).

Per-core dataflow (fp32 PSUM accumulation):
  xh/xl  = host-split x.T e4m3 hi/lo, plain linear DMA loads [d=128, o, t]
  QT_h = wq.T @ x.T  (PE DR split3, accum over d)  [dk=128, t] fp16
  KT   = wk.T @ x.T  (DR split3)                   [dk=128, t] fp16
  V    = x @ wv      (DR split3, natural)          [t=128-blk, dk] fp16
  RoPE: Q batched over the 4 heads; K per slice (fp16 DVE).
  per qslice j (512 queries), head h, key block tkb<=4j+3:
    ST  = KT_blk.T-contraction QK matmul -> PSUM [tk=128, tq=512] fp16 ops
    PT  = exp(scale*ST) on ACT -> SBUF fp16; diag blocks masked by 0/1 mult
          (alternating DVE/Pool)
    yt += V_blk.T @ PT   (PE accum)          [dk=128, tq=512]
  den = ones128.T @ tree_sum(PT blocks) (DVE tree, 1 PE matmul)
  ysb = yt * recip(den) (DVE) fp16; yhi = e4m3 (ACT copy);
  ylo = ysb - yhi (DVE) e4m3
  out[tq,:] += (partial) sum_h ysb_h.T @ wo_h  (PE DR split3 over head
  pairs, accum over terms; fp16 out, D scale folded out on host)

Schedule: token slice 0's projections run upfront with column-priority
DMA loads (ts0 x columns + wq stream in first, so the PE starts ~2us in).
Attention j's blocks are the emission spine; a pump drains filler units
between blocks — slice j+1's Q/rope (and K/V, deferred into early j+1
where legal since j+1 reads those K/V blocks only in its last pairs) and
wo_stage(j-1) groups — so the in-order PE stream always has independent
work while the exp chain trails, and ACT/exp work spreads over the whole
timeline. Per-head finalize (den matmul / recip / ysb / e4m3 hi-lo) is
deferred two blocks into the next head (and across the slice boundary
for the last head) so it never fills the 4-deep PE wait queue; the very
last finalize runs in column halves, with its den accumulated directly
over the ptq partial sums, so wo_final's first groups start sooner.

Error budget spend: the Q projection drops 3 of its 16 (term,
chunk-pair) corrections (lh pair 7, hl pairs 5-6) — each dropped pair
re-admits ~0.9e-2 of independent quantization noise (quadrature-
additive, model matches measurement within 5%) and saves 1.7us of PE.
Total rel err 1.58e-2 vs the 2e-2 gate (21% margin).

Engine balance: PE ~159us busy / 172.6us span, ACT ~132 (exp +
projection-PSUM copies + yhi), DVE ~126 (rope, tree-sums, mask-evens,
recip/ysb, wo copies), Pool ~88 (mask-odds, level-1 tree head + j2/j3
extras, ylo, j3 yhi, DMA queues). The ACT exp table is pre-warmed
during the projection phase.
"""

import sys
from contextlib import ExitStack

import numpy as np

sys.path.insert(0, "/opt/trn_rl_repo")

import ml_dtypes

import bass_rust
import concourse.bass as bass
import concourse.mybir as mybir
import concourse.tile as tile
from concourse.bass_utils import run_bass_kernel_spmd

B, T, D = 2, 2048, 2048
H, G, DK = 16, 4, 128
HPC = H // G          # 4 query heads per core
P = 128
NDC = D // P          # 16 contraction chunks
NPR = NDC // 2        # 8 DoubleRow chunk pairs
NTB = T // P          # 16 token blocks
QS = 512              # query slice (matmul moving dim)
NQS = T // QS         # 4
ND = D // QS          # 4 output column slices
THETA = 10000.0
SW = float(np.sqrt(D))                    # weight pre-scale (host)
SCALE = 1.0 / (float(np.sqrt(DK)) * D)    # exp scale: 1/sqrt(dk) / SW^2
OUTSCALE = 1.0 / D                        # host-side final unscale (SW^2)
F16 = mybir.dt.float16
E4 = mybir.dt.float8e4
F32 = mybir.dt.float32
DR = mybir.MatmulPerfMode.DoubleRow

_CACHE = {}
_NSPLIT = [0]


def split_multi_waits(nc):
    """Walrus codegen accepts at most one sem wait per instruction; Tile's
    sem assignment can emit several. Hoist extras onto single-wait NOPs
    inserted immediately before, on the same engine stream."""
    n = 0
    for f in nc.m.functions:
        for b in f.blocks:
            insts = b.instructions
            newl = []
            changed = False
            for ins in insts:
                si = getattr(ins, "sync_info", None)
                if si is not None and si.on_wait and len(si.on_wait) > 1:
                    waits = list(si.on_wait)
                    for w in waits[:-1]:
                        _NSPLIT[0] += 1
                        nop = bass_rust.InstNoOp(
                            name=f"I-wsplit{_NSPLIT[0]}",
                            engine=ins.engine,
                            ins=[], outs=[],
                            bass_nofuse=True,
                            sync_info=mybir.SyncInfo(on_wait=[w], on_update=[]),
                        )
                        newl.append(nop)
                        n += 1
                    ins.sync_info = mybir.SyncInfo(
                        on_wait=[waits[-1]], on_update=list(si.on_update or [])
                    )
                    changed = True
                newl.append(ins)
            if changed:
                insts.clear()
                insts.extend(newl)
    return n


def build_nc():
    nc = bass.Bass()
    xh = nc.declare_dram_parameter("xh", [D, T], E4, isOutput=False)
    xl = nc.declare_dram_parameter("xl", [D, T], E4, isOutput=False)
    wqh = nc.declare_dram_parameter("wqh", [D, HPC * DK], E4, isOutput=False)
    wql = nc.declare_dram_parameter("wql", [D, HPC * DK], E4, isOutput=False)
    wkh = nc.declare_dram_parameter("wkh", [D, DK], E4, isOutput=False)
    wkl = nc.declare_dram_parameter("wkl", [D, DK], E4, isOutput=False)
    wvh = nc.declare_dram_parameter("wvh", [D, DK], E4, isOutput=False)
    wvl = nc.declare_dram_parameter("wvl", [D, DK], E4, isOutput=False)
    woh = nc.declare_dram_parameter("woh", [HPC * DK, D], E4, isOutput=False)
    wol = nc.declare_dram_parameter("wol", [HPC * DK, D], E4, isOutput=False)
    cosf = nc.declare_dram_parameter("cosf", [P, T], F16, isOutput=False)
    sinf = nc.declare_dram_parameter("sinf", [P, T], F16, isOutput=False)
    dmask = nc.declare_dram_parameter("dmask", [HPC, P, QS], F16,
                                      isOutput=False)
    out = nc.declare_dram_parameter("out", [T, D], F16, isOutput=True)

    with ExitStack() as ctx:
        tc = ctx.enter_context(tile.TileContext(nc))
        const = ctx.enter_context(tc.tile_pool(name="const", bufs=1))
        work = ctx.enter_context(tc.tile_pool(name="work", bufs=3))
        ptp = ctx.enter_context(tc.tile_pool(name="ptp", bufs=8))
        pos_ = ctx.enter_context(tc.tile_pool(name="pos_", bufs=6))
        pst = ctx.enter_context(tc.tile_pool(name="pst", bufs=3, space="PSUM"))
        pyt = ctx.enter_context(tc.tile_pool(name="pyt", bufs=2, space="PSUM"))
        pden = ctx.enter_context(tc.tile_pool(name="pden", bufs=1, space="PSUM"))
        pmm = ctx.enter_context(tc.tile_pool(name="pmm", bufs=2, space="PSUM"))

        # ---- persistent SBUF loads ----
        # Queue spreading: SP carries xh (first matmul dep) then wo, ACT
        # carries xl, Pool carries the other weights+tables in need-order.
        xh_sb = const.tile([P, NDC, T], E4, tag="xh")
        xl_sb = const.tile([P, NDC, T], E4, tag="xl")
        wqh_sb = const.tile([P, NDC, HPC * DK], E4, tag="wqh")
        wql_sb = const.tile([P, NDC, HPC * DK], E4, tag="wql")
        wkh_sb = const.tile([P, NDC, DK], E4, tag="wkh")
        wkl_sb = const.tile([P, NDC, DK], E4, tag="wkl")
        wvh_sb = const.tile([P, NDC, DK], E4, tag="wvh")
        wvl_sb = const.tile([P, NDC, DK], E4, tag="wvl")
        xh_r = xh.rearrange("(o p) t -> p o t", p=P)
        xl_r = xl.rearrange("(o p) t -> p o t", p=P)
        # Column-priority loads: ts0's x columns (0:512, all 16 chunks) land
        # first in three transfers per tensor so the PE starts ~2us in; the
        # remaining columns stream behind. Queue split: SP=xh(+wo later),
        # ACT=xl, Pool=weights/tables in first-use order.
        wqh_r = wqh.rearrange("(o p) m -> p o m", p=P)
        wql_r = wql.rearrange("(o p) m -> p o m", p=P)
        # Column-priority loads: ts0's x columns (0:512, all 16 chunks)
        # land early so the PE starts ~2.5us in; wq (first consumer) heads
        # the Pool queue in hi/lo chunk-priority order.
        nc.gpsimd.dma_start(wqh_sb[:, :4, :], wqh_r[:, :4, :])
        nc.gpsimd.dma_start(wqh_sb[:, 4:, :], wqh_r[:, 4:, :])
        nc.sync.dma_start(xh_sb[:, :4, :QS], xh_r[:, :4, :QS])
        nc.scalar.dma_start(xl_sb[:, :4, :QS], xl_r[:, :4, :QS])
        nc.sync.dma_start(xh_sb[:, 4:, :QS], xh_r[:, 4:, :QS])
        nc.scalar.dma_start(wql_sb[:, :4, :], wql_r[:, :4, :])
        nc.scalar.dma_start(xl_sb[:, 4:, :QS], xl_r[:, 4:, :QS])
        nc.sync.dma_start(wql_sb[:, 4:, :], wql_r[:, 4:, :])
        nc.sync.dma_start(xh_sb[:, :, QS:], xh_r[:, :, QS:])
        nc.scalar.dma_start(xl_sb[:, :, QS:], xl_r[:, :, QS:])
        nc.gpsimd.dma_start(wkh_sb[:], wkh.rearrange("(o p) m -> p o m", p=P))
        nc.gpsimd.dma_start(wkl_sb[:], wkl.rearrange("(o p) m -> p o m", p=P))
        nc.gpsimd.dma_start(wvh_sb[:], wvh.rearrange("(o p) m -> p o m", p=P))
        nc.gpsimd.dma_start(wvl_sb[:], wvl.rearrange("(o p) m -> p o m", p=P))
        # cos/sin feed the ts0 rope; mask feeds attention j0
        cos_sb = const.tile([P, T], F16, tag="cos")
        nc.gpsimd.dma_start(cos_sb[:], cosf[:])
        sin_sb = const.tile([P, T], F16, tag="sin")
        nc.gpsimd.dma_start(sin_sb[:], sinf[:])
        mask_sb = const.tile([P, HPC, QS], F16, tag="mask")
        nc.gpsimd.dma_start(mask_sb[:], dmask.rearrange("d p q -> p d q"))
        # wo isn't read until the first wo_stage (~60% in): ride SP after xh
        woh_sb = const.tile([P, HPC, D], E4, tag="woh")
        nc.sync.dma_start(woh_sb[:], woh.rearrange("(h p) n -> p h n", p=P))
        wol_sb = const.tile([P, HPC, D], E4, tag="wol")
        nc.sync.dma_start(wol_sb[:], wol.rearrange("(h p) n -> p h n", p=P))
        ones_sb = const.tile([P, P], F16, tag="ones")
        nc.vector.memset(ones_sb[:], 1.0)
        # zero-init the pt pool slots: diagonal blocks only exp the unmasked
        # columns, and mask*stale-NaN would poison the sums otherwise
        for i in range(12):
            ptz = ptp.tile([P, QS], F16, tag="pt", name=f"ptz{i}", bufs=12)
            nc.vector.memset(ptz[:], 0.0)
        # warm the ACT exp table during the (ACT-idle) projection phase
        warm = work.tile([P, 1], F32, tag="warm", name="warm")
        nc.vector.memset(warm[:], 0.0)
        nc.scalar.activation(warm[:], warm[:],
                             mybir.ActivationFunctionType.Exp)

        # ---- projections (sliced, interleaved with attention) ----
        _pp = [(pmm, "mm"), (pst, "st"), (pyt, "yt"), (pden, "den")]
        _pg = [0]

        def proj_psum(cyc):
            if cyc:
                pool, tg = _pp[_pg[0] % 4]
                _pg[0] += 1
            else:
                pool, tg = pmm, "mm"
            return pool.tile([P, QS], F32, tag=tg, name=f"pj{_pg[0]}_{cyc}")

        def dr_term(ps, csl, w_ap, x_ap, start, stop):
            """One hi/lo term: 8 DoubleRow chunk-pair matmuls into ps[:, csl].
            w_ap: [P, NDC, M<=128] stationary; x_ap: [P, NDC, N<=256]
            moving (column slice pre-applied)."""
            for pr in range(NPR):
                nc.tensor.matmul(
                    ps[:, csl],
                    w_ap[:, 2 * pr:2 * pr + 2, :],
                    x_ap[:, 2 * pr:2 * pr + 2, :],
                    start=(start and pr == 0),
                    stop=(stop and pr == NPR - 1),
                    perf_mode=DR,
                )

        def dr3(ps, csl, wh_ap, wl_ap, xh_ap, xl_ap, skip_lh=(),
                skip_hl=()):
            """3-term hi/lo DoubleRow projection into ps[:, csl].
            skip_lh/skip_hl drop the correction terms for those chunk
            pairs: each dropped pair adds ~1e-2*sqrt(1/8) relative noise
            to this projection's output — spent deliberately from the
            error budget (gate 2e-2, floor 2.5e-3) to save PE cycles."""
            plan = [(wh_ap, xh_ap, pr) for pr in range(NPR)]
            plan += [(wl_ap, xh_ap, pr) for pr in range(NPR)
                     if pr not in skip_lh]
            plan += [(wh_ap, xl_ap, pr) for pr in range(NPR)
                     if pr not in skip_hl]
            for i, (w_ap, x_ap, pr) in enumerate(plan):
                nc.tensor.matmul(
                    ps[:, csl],
                    w_ap[:, 2 * pr:2 * pr + 2, :],
                    x_ap[:, 2 * pr:2 * pr + 2, :],
                    start=(i == 0), stop=(i == len(plan) - 1),
                    perf_mode=DR,
                )

        def rope_slice(dst, ts, nm):  # dst: [128, 512] f16 AP, token slice ts
            sl = slice(ts * QS, (ts + 1) * QS)
            sw = work.tile([P, QS], F16, tag="swp", name=f"sw{nm}")
            nc.gpsimd.dma_start(sw[0:64, :], dst[64:128, :])
            nc.gpsimd.dma_start(sw[64:128, :], dst[0:64, :])
            nc.vector.tensor_mul(sw[:], sw[:], sin_sb[:, sl])
            nc.vector.tensor_mul(dst, dst, cos_sb[:, sl])
            nc.vector.tensor_add(dst, dst, sw[:])

        QT = const.tile([P, HPC, T], F16, tag="QT")
        KT = const.tile([P, T], F16, tag="KT")
        Vn = const.tile([P, NTB, DK], F16, tag="Vn")

        def proj_q_units(h, ts, cyc=False):
            # two ~1.3us PE units sharing one PSUM tile (kept adjacent in
            # the filler list so the pmm ring never clobbers an open group)
            hs = slice(h * DK, (h + 1) * DK)
            t0 = ts * QS
            box = [None]

            def unit_a():
                box[0] = proj_psum(cyc)
                dr3(box[0], slice(0, 256),
                    wqh_sb[:, :, hs], wql_sb[:, :, hs],
                    xh_sb[:, :, t0:t0 + 256], xl_sb[:, :, t0:t0 + 256],
                    skip_lh=(7,), skip_hl=(5, 6))

            def unit_b():
                dr3(box[0], slice(256, 512),
                    wqh_sb[:, :, hs], wql_sb[:, :, hs],
                    xh_sb[:, :, t0 + 256:t0 + QS],
                    xl_sb[:, :, t0 + 256:t0 + QS],
                    skip_lh=(7,), skip_hl=(5, 6))
                nc.scalar.copy(QT[:, h, t0:t0 + QS], box[0][:])

            return [unit_a, unit_b]

        def rope_q4(ts):
            # batched rope over all 4 heads of a query slice
            sl = slice(ts * QS, (ts + 1) * QS)
            qs = QT[:, :, sl]
            sw = work.tile([P, HPC, QS], F16, tag="sw4", name=f"sw4_{ts}", bufs=2)
            nc.gpsimd.dma_start(sw[0:64, :, :], QT[64:128, :, sl])
            nc.gpsimd.dma_start(sw[64:128, :, :], QT[0:64, :, sl])
            sinb = sin_sb[:, sl].rearrange(
                "p (o c) -> p o c", o=1).broadcast_to((P, HPC, QS))
            cosb = cos_sb[:, sl].rearrange(
                "p (o c) -> p o c", o=1).broadcast_to((P, HPC, QS))
            nc.vector.tensor_mul(sw[:], sw[:], sinb)
            nc.vector.tensor_mul(qs, qs, cosb)
            nc.vector.tensor_add(qs, qs, sw[:])

        def proj_k_units(ts, cyc=False):
            t0 = ts * QS
            box = [None]

            def unit_a():
                box[0] = proj_psum(cyc)
                dr3(box[0], slice(0, 256), wkh_sb, wkl_sb,
                    xh_sb[:, :, t0:t0 + 256], xl_sb[:, :, t0:t0 + 256])

            def unit_b():
                dr3(box[0], slice(256, 512), wkh_sb, wkl_sb,
                    xh_sb[:, :, t0 + 256:t0 + QS],
                    xl_sb[:, :, t0 + 256:t0 + QS])
                nc.scalar.copy(KT[:, t0:t0 + QS], box[0][:])
                rope_slice(KT[:, t0:t0 + QS], ts, f"k{ts}")

            return [unit_a, unit_b]

        def proj_v_unit(tb, cyc=False):
            # natural-layout V: lhsT = x chunk pairs (stationary),
            # rhs = wv chunk pairs; out [t-block 128, dk 128]
            def unit():
                ps = proj_psum(cyc)
                tsl = slice(tb * P, (tb + 1) * P)
                first = True
                for (xt_, wt) in ((xh_sb, wvh_sb), (xl_sb, wvh_sb),
                                  (xh_sb, wvl_sb)):
                    for pr in range(NPR):
                        nc.tensor.matmul(
                            ps[:, :DK],
                            xt_[:, 2 * pr:2 * pr + 2, tsl],
                            wt[:, 2 * pr:2 * pr + 2, :],
                            start=first,
                            stop=(xt_ is xh_sb and wt is wvl_sb
                                  and pr == NPR - 1),
                            perf_mode=DR,
                        )
                        first = False
                nc.scalar.copy(Vn[:, tb, :], ps[:, :DK])

            return unit

        def proj_slice_units(ts, cyc=False):
            """All projection work for token slice ts, as filler units.
"""
            units = []
            for h in range(HPC):
                units += proj_q_units(h, ts, cyc=cyc)
            units.append(lambda: rope_q4(ts))
            units += proj_k_units(ts, cyc=cyc)
            for tb in range(4 * ts, 4 * ts + 4):
                units.append(proj_v_unit(tb, cyc=cyc))
            return units

        # token slice 0 runs upfront (attention j0 needs it); slices j+1
        # are spread through attention j as PE filler. ts0's Q projection
        # is emitted TERM-major (all hh chains, then lh, then hl) across
        # the four cycled PSUM pools, so wqh/wql/xl are each needed as
        # late as possible while their DMAs stream in.
        for u in proj_slice_units(0, cyc=True):
            u()

        _oq = [0]

        def wo_group(j, yhi, ylo, tqb, ds, pool, tg, pw, off, cpe, eng,
                     osb_sh=None):
            # one output-projection group: DR split3 over head pairs
            # (contraction = 4 heads x 128 = 2 DoubleRow pairs x 3 terms,
            # per 256-col chunk), PSUM->SBUF copy, out-DMA
            r0 = j * QS + tqb * P
            tq = slice(tqb * P, (tqb + 1) * P)
            po = pool.tile([P, QS], F32, tag=tg,
                           name=f"po{j}_{tqb}_{ds}_{off}")
            for cc in range(pw // 256):
                pc0 = cc * 256
                wsl = slice(ds * QS + off + pc0, ds * QS + off + pc0 + 256)
                first = True
                for (yt_, wt) in ((yhi, woh_sb), (ylo, woh_sb),
                                  (yhi, wol_sb)):
                    for hp in range(HPC // 2):
                        nc.tensor.matmul(
                            po[:, pc0:pc0 + 256],
                            yt_[:, 2 * hp:2 * hp + 2, tq],
                            wt[:, 2 * hp:2 * hp + 2, wsl],
                            start=first,
                            stop=(yt_ is yhi and wt is wol_sb
                                  and hp == HPC // 2 - 1),
                            perf_mode=DR,
                        )
                        first = False
            if osb_sh is not None:
                osb = osb_sh[:, off:off + pw]
            else:
                osb = pos_.tile([P, QS], F16, tag="osb",
                                name=f"osb{j}_{tqb}_{ds}_{off}")
                osb = osb[:, :pw]
            if cpe is nc.scalar:
                nc.scalar.copy(osb[:], po[:, :pw])
            else:
                cpe.tensor_copy(osb[:], po[:, :pw])
            eng.dma_start(
                out[r0:r0 + P, ds * QS + off:ds * QS + off + pw], osb[:])

        def wo_units(j, yhi, ylo):
            units = []
            for tqb in range(QS // P):
                for ds in range(ND):
                    def unit(tqb=tqb, ds=ds):
                        cpe = nc.vector
                        eng = nc.sync if _oq[0] % 2 == 0 else nc.gpsimd
                        _oq[0] += 1
                        wo_group(j, yhi, ylo, tqb, ds, pmm, "mm",
                                 QS, 0, cpe, eng)
                    unit.is_wo = True
                    units.append(unit)
            return units

        def wo_final(j, yhi, ylo):
            # final stage: attention PSUM pools are free — cycle po across
            # all four; split the very last group into two half-width
            # pieces on different banks so the copy+DMA drain pipelines
            gi = 0
            for tqb in range(QS // P):
                for ds in range(ND):
                    last = tqb == QS // P - 1 and ds >= ND - 2
                    if not last:
                        pool, tg = _pp[gi % 4]
                        gi += 1
                        cpe = nc.scalar if gi % 2 == 0 else nc.vector
                        eng = nc.sync if _oq[0] % 2 == 0 else nc.gpsimd
                        _oq[0] += 1
                        wo_group(j, yhi, ylo, tqb, ds, pool, tg,
                                 QS, 0, cpe, eng)
                    else:
                        osb_sh = pos_.tile([P, QS], F16, tag="osb",
                                           name=f"osbshf{j}_{ds}")
                        for pc in range(2):
                            pool, tg = _pp[gi % 4]
                            gi += 1
                            wo_group(j, yhi, ylo, tqb, ds, pool, tg,
                                     256, pc * 256,
                                     [nc.scalar, nc.vector][pc],
                                     [nc.gpsimd, nc.sync][pc], osb_sh=osb_sh)

        def proj_kv_units(ts, cyc=False):
            units = proj_k_units(ts, cyc=cyc)
            for tb in range(4 * ts, 4 * ts + 4):
                units.append(proj_v_unit(tb, cyc=cyc))
            return units

        def proj_qr_units(ts, cyc=False):
            units = []
            for h in range(HPC):
                units += proj_q_units(h, ts, cyc=cyc)
            units.append(lambda: rope_q4(ts))
            return units

        ysbs = {}
        fin = [None]   # deferred per-head finalize (crosses slice bounds)
        # ---- attention + output projection, per query slice ----
        # Emission interleave: attention j's blocks are the spine; between
        # blocks a pump drains filler units (projection slice j+1 and
        # wo_stage(j-1) groups) so the in-order PE stream always has
        # independent work while the exp chain trails, and ACT/exp work is
        # spread over the whole timeline instead of piling up after the
        # projection phase.
        for j in range(NQS):
            prio = []   # must complete early: drained one per block
            if fin[0] is not None:
                prio.append(fin[0])
                fin[0] = None
            fillers = []
            if j == NQS - 1:
                prio += proj_kv_units(j)
            if j + 1 < NQS:
                fillers += proj_qr_units(j + 1)
                if j + 1 < NQS - 1:
                    fillers += proj_kv_units(j + 1)
            if j >= 1:
                fillers += wo_units(j - 1, *ysbs[j - 1])
            nunits = len(fillers)
            nblocks = HPC * (4 * j + 4)
            emitted = [0]

            def pump(bdone):
                if prio:
                    prio.pop(0)()
                want = nunits * bdone // nblocks
                while emitted[0] < want and fillers:
                    fillers.pop(0)()
                    emitted[0] += 1

            ysb = work.tile([P, HPC, QS], F16, tag="ysb", bufs=2)
            yhi = work.tile([P, HPC, QS], E4, tag="yhi", bufs=2)
            ylo = work.tile([P, HPC, QS], E4, tag="ylo", bufs=2)
            nkb = 4 * j + 4  # causal: key blocks 0..4j+3
            for h in range(HPC):
                yt = pyt.tile([P, QS], F32, tag="yt")
                prev_pt = None
                ptot = None
                ptqs = []
                for tkb in range(nkb):
                    pump(h * nkb + tkb)
                    if tkb == 2 and fin[0] is not None:
                        # emit the previous head's finalize a couple of
                        # blocks in, so its den-matmul + the new head's
                        # first AVs don't fill the 4-deep PE wait queue at
                        # the head boundary
                        fin[0]()
                        fin[0] = None
                    d = tkb - 4 * j
                    # columns left of 128*d are fully masked for diagonal
                    # blocks: skip them in QK/exp/AV; the mask-mult zeroes
                    # the stale region of pt so den/AV sums stay exact.
                    c0 = max(d, 0) * P
                    st = pst.tile([P, QS], F32, tag="st")
                    nc.tensor.matmul(
                        st[:, c0:],
                        KT[:, tkb * P:(tkb + 1) * P],
                        QT[:, h, j * QS + c0:(j + 1) * QS],
                        start=True, stop=True,
                    )
                    pt = ptp.tile([P, QS], F16, tag="pt", bufs=12)
                    nc.scalar.activation(
                        pt[:, c0:], st[:, c0:],
                        mybir.ActivationFunctionType.Exp, scale=SCALE,
                    )
                    if d >= 0:
                        # full-width 0/1 mask also zeroes the stale c0
                        # region; alternate DVE/Pool to balance load
                        if d % 2 == 0:
                            nc.vector.tensor_mul(pt[:], pt[:],
                                                 mask_sb[:, d, :])
                        else:
                            nc.gpsimd.tensor_mul(pt[:], pt[:],
                                                 mask_sb[:, d, :])
                    nc.tensor.matmul(
                        yt[:, c0:], Vn[:, tkb, :], pt[:, c0:],
                        start=(tkb == 0), stop=(tkb == nkb - 1),
                    )
                    # denominator: tree-sum all PT blocks on DVE, then one
                    # ones-matmul per (h, j) for the partition reduction
                    if tkb % 2 == 0:
                        prev_pt = pt
                    else:
                        pts = ptp.tile([P, QS], F16, tag="pts",
                                       name=f"pts{j}_{h}_{tkb}", bufs=4)
                        # first tree level alternates Pool/DVE: off-DVE adds
                        # keep pt-slot recycling from serializing behind the
                        # per-head finalize, while capping Pool load
                        if (tkb <= 3 or (j == NQS - 1 and tkb <= 7)
                                or (j == NQS - 2 and tkb <= 5)):
                            nc.gpsimd.tensor_add(pts[:], prev_pt[:], pt[:])
                        else:
                            nc.vector.tensor_add(pts[:], prev_pt[:], pt[:])
                        if tkb % 4 == 1:
                            prev_pts = pts
                        else:
                            ptq = ptp.tile([P, QS], F16, tag="ptq", bufs=5,
                                           name=f"ptq{j}_{h}_{tkb}")
                            nc.vector.tensor_add(ptq[:], prev_pts[:], pts[:])
                            if j == NQS - 1 and h == HPC - 1:
                                ptqs.append(ptq)
                            elif ptot is None:
                                ptot = ptq
                            else:
                                nxt = ptp.tile([P, QS], F16, tag="ptt",
                                               bufs=5,
                                               name=f"ptt{j}_{h}_{tkb}")
                                nc.vector.tensor_add(nxt[:], ptot[:], ptq[:])
                                ptot = nxt
                def finalize(h=h, yt=yt, ptot=ptot, ptqs=ptqs, j=j,
                             ysb=ysb, yhi=yhi, ylo=ylo):
                    den = pden.tile([P, QS], F32, tag="den",
                                    name=f"den{j}_{h}")
                    recipb = work.tile([P, QS], F32, tag="recipb",
                                       name=f"rb{j}_{h}", bufs=2)
                    lastf = j == NQS - 1 and h == HPC - 1
                    # the very last finalize feeds wo_final directly: run
                    # it in column halves so the first wo groups start
                    # sooner
                    halves = ((slice(0, 256), slice(256, QS)) if lastf
                              else (slice(0, QS),))
                    for cs in halves:
                        if ptqs:
                            for qi, pq in enumerate(ptqs):
                                nc.tensor.matmul(
                                    den[:, cs], ones_sb[:], pq[:, cs],
                                    start=(qi == 0),
                                    stop=(qi == len(ptqs) - 1))
                        else:
                            nc.tensor.matmul(den[:, cs], ones_sb[:],
                                             ptot[:, cs],
                                             start=True, stop=True)
                        nc.vector.reciprocal(recipb[:, cs], den[:, cs])
                        nc.vector.tensor_mul(ysb[:, h, cs], yt[:, cs],
                                             recipb[:, cs])
                        if lastf:
                            # short critical path into wo_final: keep DVE
                            nc.vector.tensor_copy(yhi[:, h, cs],
                                                  ysb[:, h, cs])
                        elif j == NQS - 1:
                            # j3: ACT is exp-bound and DVE is saturated
                            nc.gpsimd.tensor_copy(yhi[:, h, cs],
                                                  ysb[:, h, cs])
                        else:
                            nc.scalar.copy(yhi[:, h, cs], ysb[:, h, cs])
                        if lastf:
                            nc.vector.tensor_sub(ylo[:, h, cs],
                                                 ysb[:, h, cs],
                                                 yhi[:, h, cs])
                        else:
                            nc.gpsimd.tensor_sub(ylo[:, h, cs],
                                                 ysb[:, h, cs],
                                                 yhi[:, h, cs])

                fin[0] = finalize

            if j == NQS - 1 and fin[0] is not None:
                # last slice: wo_final needs the final head's outputs now
                fin[0]()
                fin[0] = None
            while fillers:   # drain leftovers before the next slice
                fillers.pop(0)()
            ysbs[j] = (yhi, ylo)
        wo_final(NQS - 1, *ysbs[NQS - 1])
    split_multi_waits(nc)
    return nc


def _rope_tables(pos):
    inv_freq = 1.0 / (THETA ** (np.arange(0, DK // 2, dtype=np.float64) * 2.0 / DK))
    ang = pos.astype(np.float64)[:, None] * inv_freq[None, :]   # (T, 64)
    cos = np.cos(ang).T.astype(np.float32)                      # (64, T)
    sin = np.sin(ang).T.astype(np.float32)
    cosf = np.concatenate([cos, cos], axis=0)                   # (128, T)
    sinf = np.concatenate([-sin, sin], axis=0)
    return cosf, sinf


def _e4_split(a):
    """Split fp32 array into e4m3 hi + lo (residual)."""
    e4 = ml_dtypes.float8_e4m3
    hi = a.astype(e4)
    lo = (a - hi.astype(np.float32)).astype(e4)
    return hi, lo


def _make_in_maps(inputs):
    x, Wq, Wk, Wv, Wo = (np.asarray(inputs[k], dtype=np.float32) for k in
                         ("x", "Wq", "Wk", "Wv", "Wo"))
    f16 = np.float16
    cosf, sinf = _rope_tables(np.asarray(inputs["pos"]))
    cosf = cosf.astype(f16)
    sinf = sinf.astype(f16)
    # diagonal-region 0/1 masks: dmask[d][tk, tq] = mask[tq, d*128 + tk]
    m = np.asarray(inputs["mask"])
    dmask = np.stack(
        [m[0:QS, d * P:(d + 1) * P].T for d in range(HPC)], axis=0
    ).astype(f16)

    in_maps = []
    for c in range(8):
        b, g = c // 4, c % 4
        xt = np.ascontiguousarray(x[b].T)
        xh_, xl_ = _e4_split(xt)
        wq_h, wq_l = _e4_split(Wq[:, g * HPC * DK:(g + 1) * HPC * DK] * SW)
        wk_h, wk_l = _e4_split(Wk[:, g * DK:(g + 1) * DK] * SW)
        wv_h, wv_l = _e4_split(Wv[:, g * DK:(g + 1) * DK] * SW)
        wo_h, wo_l = _e4_split(Wo[g * HPC * DK:(g + 1) * HPC * DK, :] * SW)
        in_maps.append({
            "xh": xh_, "xl": xl_,
            "wqh": wq_h, "wql": wq_l,
            "wkh": wk_h, "wkl": wk_l,
            "wvh": wv_h, "wvl": wv_l,
            "woh": wo_h, "wol": wo_l,
            "cosf": cosf, "sinf": sinf, "dmask": dmask,
        })
    return in_maps


def kernel(x, Wq, Wk, Wv, Wo, mask, pos):
    in_maps = _make_in_maps(dict(x=x, Wq=Wq, Wk=Wk, Wv=Wv, Wo=Wo,
                                 mask=mask, pos=pos))
    if "nc" not in _CACHE:
        _CACHE["nc"] = build_nc()
    nc = _CACHE["nc"]

    res = run_bass_kernel_spmd(nc, in_maps, core_ids=list(range(8)))
    outs = [np.asarray(r["out"], dtype=np.float32) for r in res.results]
    full = np.stack([
        outs[0] + outs[1] + outs[2] + outs[3],
        outs[4] + outs[5] + outs[6] + outs[7],
    ]).astype(np.float32) * np.float32(OUTSCALE)
    return full
